# revision 10
# baseline (speedup 1.0000x reference)
"""GNN message-passing layer (LplsNorm + residual conv) on 8 Trainium2 cores.

Computation (reference, all f32):
    degree = A.sum(-1); ds = degree**-0.5
    mf  = f + ds[:,None] * (A @ (ds[:,None] * f))      # a_norm = ds A ds
    out = relu(mf @ W + b)

Distribution: A row-sharded over 8 cores ([1024, 8192] each), feature
replicated (each core loads full f from its own HBM copy).

Per-core schedule (v5):
  Phase 1 (DMA-bound): stream A shard in [128, 2048] f32 chunks.
    Per chunk: ScalarE casts to bf16, GpSimd accumulates row sums
    (degree), TensorE transposes the 16 bf16 tiles (matmul-with-
    identity), DVE copies all transposed groups PSUM->SBUF. ALL of A^T
    stays resident in SBUF (128 KiB/partition) -- no DRAM spill.
    fres/W/bias load on the second HWDGE ring (scalar), bias broadcast
    via one K=1 matmul.
  Phase 2: ds = 1/sqrt(degree) computed LOCALLY, tiny AllGather of ds;
    ~40 junk matmuls keep the PE HAM-warm across the collective stall.
  Phase 3 (PE-bound): kc-outer matmul over 2 groups of 4 m-tiles.
    Per kc pair: one 512 KiB f DMA, DVE scale+cast to bf16 xp, 8
    back-to-back N=512 bf16 matmuls. Epilogue per m-tile: mf = ds*Y +
    fres (DVE stt), mf @ W in f32r, bias add via pre-broadcast tile,
    ACT relu, output DMA on the scalar ring.
"""

import numpy as np

import concourse.bass as bass
import concourse.mybir as mybir
import concourse.tile as tile
from concourse import bacc
from concourse import bass_utils
from concourse.masks import make_identity

N = 8192
D = 512
NCORES = 8
P = 128
R = N // NCORES          # rows per core: 1024
MT = R // P              # m-tiles per core: 8
KC = N // P              # k-chunks: 64
ACH = 2048               # A stream chunk width (f32 -> 1 MiB per DMA)
NACH = N // ACH          # stream chunks per row-block: 4
GPC = ACH // (4 * P)     # transpose groups (of 4 tiles) per stream chunk: 4
MTG = 4                  # m-tiles per matmul group (PSUM accumulators)
FKC = 2                  # k-chunks per f-stream DMA (512 KiB)
NWARM = 40               # junk matmuls bridging the collective stall

F32 = mybir.dt.float32
F32R = mybir.dt.float32r
BF16 = mybir.dt.bfloat16

_NC_CACHE = {}


def _build():
    nc = bacc.Bacc("TRN2", target_bir_lowering=False, debug=False, num_devices=NCORES)

    a_d = nc.dram_tensor("a", [R, N], F32, kind="ExternalInput")
    f_d = nc.dram_tensor("f", [N, D], F32, kind="ExternalInput")
    fres_d = nc.dram_tensor("fres", [R, D], F32, kind="ExternalInput")
    w_d = nc.dram_tensor("w", [D, D], F32R, kind="ExternalInput")
    b_d = nc.dram_tensor("bias", [1, D], F32, kind="ExternalInput")
    out_d = nc.dram_tensor("out", [R, D], F32, kind="ExternalOutput")

    AX = mybir.AxisListType.X
    ALU = mybir.AluOpType
    ACT = mybir.ActivationFunctionType

    with tile.TileContext(nc) as tc:
        with (
            tc.tile_pool(name="const", bufs=1) as constp,
            tc.tile_pool(name="deg", bufs=1) as degp,
            tc.tile_pool(name="astream", bufs=3) as astreamp,
            tc.tile_pool(name="small", bufs=2) as smallp,
            tc.tile_pool(name="atres", bufs=1) as atresp,
            tc.tile_pool(name="fstream", bufs=3) as fstreamp,
            tc.tile_pool(name="xpp", bufs=4) as xpp,
            tc.tile_pool(name="epi", bufs=2) as epip,
            tc.tile_pool(name="mft", bufs=2) as mftp,
            tc.tile_pool(name="psA", bufs=3, space="PSUM") as psA,     # transposes + W-mm out
            tc.tile_pool(name="psY", bufs=MTG, space="PSUM") as psY,   # Y accumulators
            tc.tile_pool(name="psaux", bufs=1, space="PSUM") as psaux, # small transposes
            tc.tile_pool(name="dram", bufs=1, space="DRAM") as dramp,
        ):
            # ---- constants ----
            identity = constp.tile([P, P], F32)
            make_identity(nc, identity[:])
            identity_bf = constp.tile([P, P], BF16)
            make_identity(nc, identity_bf[:])
            ones1 = constp.tile([1, P], F32)
            nc.gpsimd.memset(ones1[:], 1.0)
            b_sb = constp.tile([1, D], F32)
            nc.scalar.dma_start(b_sb[:], b_d.ap())
            w_sb = constp.tile([P, 4 * D], F32R)  # w chunk wc at [:, wc*D:(wc+1)*D]
            for wc in range(4):
                nc.scalar.dma_start(
                    w_sb[:, wc * D : (wc + 1) * D], w_d.ap()[wc * P : (wc + 1) * P, :]
                )
            # bias broadcast [128, D] via K=1 matmul (once)
            b_ps = psA.tile([P, D], F32, tag="trp", name="b_ps")
            nc.tensor.matmul(b_ps[:], ones1[:], b_sb[:])
            b_bcast = constp.tile([P, D], F32)
            nc.vector.tensor_copy(b_bcast[:], b_ps[:])

            # fres -> bf16 resident (residual term)
            fres_bf = constp.tile([P, MT * D], BF16)
            for mt in range(MT):
                fch0 = fstreamp.tile([P, D], F32, tag="fch")
                nc.scalar.dma_start(fch0[:], fres_d.ap()[mt * P : (mt + 1) * P, :])
                nc.vector.tensor_copy(fres_bf[:, mt * D : (mt + 1) * D], fch0[:])

            # resident transposed-A store: (mt, kc) tile at col (mt*KC+kc)*P
            at_res = atresp.tile([P, MT * KC * P], BF16)
            cin = dramp.tile([MT, P], F32)
            cout = dramp.tile([KC, P], F32)

            # ---- merged pass: degree + transpose-all (A read exactly once) ----
            degree_sb = degp.tile([P, MT], F32)  # col mt = degree of rows mt*128..
            for mt in range(MT):
                dcols = smallp.tile([P, NACH], F32, tag="dcols")
                for c in range(NACH):
                    ach = astreamp.tile([P, ACH], F32, tag="ach", bufs=2)
                    nc.sync.dma_start(
                        ach[:], a_d.ap()[mt * P : (mt + 1) * P, c * ACH : (c + 1) * ACH]
                    )
                    achb = astreamp.tile([P, ACH], BF16, tag="achb", bufs=2)
                    nc.scalar.activation(
                        achb[:], ach[:], ACT.Copy, accum_out=dcols[:, c : c + 1]
                    )
                    for g in range(GPC):
                        trp = psA.tile([P, 4 * P], F32, tag="trp")
                        for q in range(4):
                            nc.tensor.matmul(
                                trp[:, q * P : (q + 1) * P],
                                achb[:, (g * 4 + q) * P : (g * 4 + q + 1) * P],
                                identity_bf[:],
                            )
                        kc0 = c * (GPC * 4) + g * 4
                        dst = at_res[
                            :, (mt * KC + kc0) * P : (mt * KC + kc0 + 4) * P
                        ]
                        nc.vector.tensor_copy(dst[:], trp[:])
                nc.vector.reduce_sum(degree_sb[:, mt : mt + 1], dcols[:], axis=AX)

            # ---- ds local, AllGather ds ----
            recip8 = degp.tile([P, MT], F32)
            nc.vector.reciprocal(recip8[:], degree_sb[:])
            dsown = degp.tile([P, MT], F32)
            nc.scalar.activation(dsown[:], recip8[:], ACT.Sqrt)
            dsT_ps = psaux.tile([MT, P], F32, tag="aux")
            nc.tensor.transpose(dsT_ps[:], dsown[:], identity[:])
            dsT_sb = smallp.tile([MT, P], F32, tag="degT")
            nc.vector.tensor_copy(dsT_sb[:], dsT_ps[:])
            nc.sync.dma_start(cin[:], dsT_sb[:])
            nc.gpsimd.collective_compute(
                "AllGather",
                ALU.bypass,
                ins=[cin.opt()],
                outs=[cout.opt()],
                replica_groups=[list(range(NCORES))],
            )
            # keep the PE HAM-warm across the collective stall
            for wi in range(NWARM):
                junk_ps = psA.tile([P, D], F32, tag="trp", name=f"junk{wi}")
                nc.tensor.matmul(junk_ps[:], identity_bf[:], at_res[:, :D])
            # cout row g = global ds of rows [g*128, (g+1)*128)
            dsall_sb = smallp.tile([KC, P], F32, tag="degall")
            nc.sync.dma_start(dsall_sb[:], cout[:])
            dsallT_ps = psaux.tile([P, KC], F32, tag="aux")
            nc.tensor.transpose(dsallT_ps[:], dsall_sb[:], identity[:KC, :KC])
            ds_sb = degp.tile([P, KC], F32)  # ds_sb[p, g] = ds[g*128 + p]
            nc.vector.tensor_copy(ds_sb[:], dsallT_ps[:])

            # ---- main matmul: kc-outer over 2 groups of MTG m-tiles ----
            f_blk = f_d.ap().rearrange("(b c p) d -> b p c d", c=FKC, p=P)
            for mtg in range(MT // MTG):
                ys = [
                    psY.tile([P, D], F32, tag="y", name=f"y{mtg}_{i}")
                    for i in range(MTG)
                ]
                for fb in range(KC // FKC):
                    fch = fstreamp.tile([P, FKC * D], F32, tag="fch")
                    nc.sync.dma_start(
                        fch[:].rearrange("p (c d) -> p c d", c=FKC), f_blk[fb]
                    )
                    for j in range(FKC):
                        kc = fb * FKC + j
                        xp = xpp.tile([P, D], BF16, tag="xp")
                        nc.vector.tensor_scalar_mul(
                            xp[:], fch[:, j * D : (j + 1) * D], ds_sb[:, kc : kc + 1]
                        )
                        for mi in range(MTG):
                            mt = mtg * MTG + mi
                            nc.tensor.matmul(
                                ys[mi][:],
                                at_res[
                                    :, (mt * KC + kc) * P : (mt * KC + kc + 1) * P
                                ],
                                xp[:],
                                start=(kc == 0),
                                stop=(kc == KC - 1),
                            )
                # epilogue per m-tile in the group
                for mi in range(MTG):
                    mt = mtg * MTG + mi
                    mf = epip.tile([P, D], F32, tag="mf")
                    nc.vector.scalar_tensor_tensor(
                        mf[:],
                        ys[mi][:],
                        dsown[:, mt : mt + 1],
                        fres_bf[:, mt * D : (mt + 1) * D],
                        op0=ALU.mult,
                        op1=ALU.add,
                    )
                    o_ps = psA.tile([P, D], F32, tag="trp", name=f"o_ps{mt}")
                    for wc in range(4):
                        mfT_ps = psaux.tile([P, P], F32, tag="aux")
                        nc.tensor.transpose(
                            mfT_ps[:], mf[:, wc * P : (wc + 1) * P], identity[:]
                        )
                        mfT_sb = mftp.tile([P, P], F32R, tag="mfT")
                        nc.vector.tensor_copy(mfT_sb[:], mfT_ps[:])
                        nc.tensor.matmul(
                            o_ps[:],
                            mfT_sb[:],
                            w_sb[:, wc * D : (wc + 1) * D],
                            start=(wc == 0),
                            stop=(wc == 3),
                        )
                    opre = epip.tile([P, D], F32, tag="mf", name="opre")
                    nc.vector.tensor_tensor(
                        opre[:], o_ps[:], b_bcast[:], op=ALU.add
                    )
                    osb = epip.tile([P, D], F32, tag="osb")
                    nc.scalar.activation(osb[:], opre[:], ACT.Relu)
                    nc.scalar.dma_start(out_d.ap()[mt * P : (mt + 1) * P, :], osb[:])

    nc.compile()
    return nc


def _get_nc():
    if "nc" not in _NC_CACHE:
        _NC_CACHE["nc"] = _build()
    return _NC_CACHE["nc"]


def run(inputs, trace=False, trace_kwargs=None):
    """Run the SPMD kernel; returns (full_output, BassKernelResults)."""
    a = np.ascontiguousarray(np.asarray(inputs["adjacency_matrix"], dtype=np.float32))
    f = np.ascontiguousarray(np.asarray(inputs["feature"], dtype=np.float32))
    w = np.ascontiguousarray(np.asarray(inputs["W"], dtype=np.float32))
    b = np.ascontiguousarray(np.asarray(inputs["b"], dtype=np.float32)).reshape(1, D)

    nc = _get_nc()
    in_maps = []
    for d in range(NCORES):
        rows = slice(d * R, (d + 1) * R)
        in_maps.append({"a": a[rows], "f": f, "fres": f[rows], "w": w, "bias": b})
    res = bass_utils.run_bass_kernel_spmd(
        nc,
        in_maps,
        core_ids=list(range(NCORES)),
        trace=trace,
        **(trace_kwargs or {}),
    )
    out = np.concatenate([r["out"] for r in res.results], axis=0)
    return out, res


def kernel(**inputs):
    out, _ = run(inputs, trace=False)
    return out


# revision 14
# speedup vs baseline: 1.0525x; 1.0525x over previous
"""GNN message-passing layer (LplsNorm + residual conv) on 8 Trainium2 cores.

Computation (reference, all f32):
    degree = A.sum(-1); ds = degree**-0.5
    mf  = f + ds[:,None] * (A @ (ds[:,None] * f))      # a_norm = ds A ds
    out = relu(mf @ W + b)

Distribution: A row-sharded over 8 cores ([1024, 8192] each), feature
replicated (each core loads full f from its own HBM copy).

Per-core schedule (v5):
  Phase 1 (DMA-bound): stream A shard in [128, 2048] f32 chunks.
    Per chunk: ScalarE casts to bf16, GpSimd accumulates row sums
    (degree), TensorE transposes the 16 bf16 tiles (matmul-with-
    identity), DVE copies all transposed groups PSUM->SBUF. ALL of A^T
    stays resident in SBUF (128 KiB/partition) -- no DRAM spill.
    fres/W/bias load on the second HWDGE ring (scalar), bias broadcast
    via one K=1 matmul.
  Phase 2: ds = 1/sqrt(degree) computed LOCALLY, tiny AllGather of ds;
    ~40 junk matmuls keep the PE HAM-warm across the collective stall.
  Phase 3 (PE-bound): kc-outer matmul over 2 groups of 4 m-tiles.
    Per kc pair: one 512 KiB f DMA, DVE scale+cast to bf16 xp, 8
    back-to-back N=512 bf16 matmuls. Epilogue per m-tile: mf = ds*Y +
    fres (DVE stt), mf @ W in f32r, bias add via pre-broadcast tile,
    ACT relu, output DMA on the scalar ring.
"""

import numpy as np

import concourse.bass as bass
import concourse.mybir as mybir
import concourse.tile as tile
from concourse import bacc
from concourse import bass_utils
from concourse.masks import make_identity

N = 8192
D = 512
NCORES = 8
P = 128
R = N // NCORES          # rows per core: 1024
MT = R // P              # m-tiles per core: 8
KC = N // P              # k-chunks: 64
ACH = 2048               # A stream chunk width (f32 -> 1 MiB per DMA)
NACH = N // ACH          # stream chunks per row-block: 4
GPC = ACH // (4 * P)     # transpose groups (of 4 tiles) per stream chunk: 4
MTG = 4                  # m-tiles per matmul group (PSUM accumulators)
FKC = 2                  # k-chunks per f-stream DMA (512 KiB)
NWARM = 120              # junk matmuls bridging the collective stall

F32 = mybir.dt.float32
F32R = mybir.dt.float32r
BF16 = mybir.dt.bfloat16

_NC_CACHE = {}


def _build():
    nc = bacc.Bacc("TRN2", target_bir_lowering=False, debug=False, num_devices=NCORES)

    a_d = nc.dram_tensor("a", [R, N], F32, kind="ExternalInput")
    f_d = nc.dram_tensor("f", [N, D], F32, kind="ExternalInput")
    fres_d = nc.dram_tensor("fres", [R, D], F32, kind="ExternalInput")
    w_d = nc.dram_tensor("w", [D, D], F32R, kind="ExternalInput")
    b_d = nc.dram_tensor("bias", [1, D], F32, kind="ExternalInput")
    out_d = nc.dram_tensor("out", [R, D], F32, kind="ExternalOutput")

    AX = mybir.AxisListType.X
    ALU = mybir.AluOpType
    ACT = mybir.ActivationFunctionType

    with tile.TileContext(nc) as tc:
        with (
            tc.tile_pool(name="const", bufs=1) as constp,
            tc.tile_pool(name="deg", bufs=1) as degp,
            tc.tile_pool(name="astream", bufs=3) as astreamp,
            tc.tile_pool(name="small", bufs=2) as smallp,
            tc.tile_pool(name="atres", bufs=1) as atresp,
            tc.tile_pool(name="fstream", bufs=3) as fstreamp,
            tc.tile_pool(name="xpp", bufs=4) as xpp,
            tc.tile_pool(name="epi", bufs=2) as epip,
            tc.tile_pool(name="mft", bufs=2) as mftp,
            tc.tile_pool(name="psA", bufs=3, space="PSUM") as psA,     # transposes + W-mm out
            tc.tile_pool(name="psY", bufs=MTG, space="PSUM") as psY,   # Y accumulators
            tc.tile_pool(name="psaux", bufs=1, space="PSUM") as psaux, # small transposes
            tc.tile_pool(name="dram", bufs=1, space="DRAM") as dramp,
        ):
            # ---- constants ----
            identity = constp.tile([P, P], F32)
            make_identity(nc, identity[:])
            identity_bf = constp.tile([P, P], BF16)
            make_identity(nc, identity_bf[:])
            ones1 = constp.tile([1, P], F32)
            nc.gpsimd.memset(ones1[:], 1.0)
            b_sb = constp.tile([1, D], F32)
            nc.scalar.dma_start(b_sb[:], b_d.ap())
            w_sb = constp.tile([P, 4 * D], F32R)  # w chunk wc at [:, wc*D:(wc+1)*D]
            for wc in range(4):
                nc.scalar.dma_start(
                    w_sb[:, wc * D : (wc + 1) * D], w_d.ap()[wc * P : (wc + 1) * P, :]
                )
            # bias broadcast [128, D] via K=1 matmul (once)
            b_ps = psA.tile([P, D], F32, tag="trp", name="b_ps")
            nc.tensor.matmul(b_ps[:], ones1[:], b_sb[:])
            b_bcast = constp.tile([P, D], F32)
            nc.vector.tensor_copy(b_bcast[:], b_ps[:])

            # resident transposed-A store: (mt, kc) tile at col (mt*KC+kc)*P
            at_res = atresp.tile([P, MT * KC * P], BF16)
            cin = dramp.tile([MT, P], F32)
            cout = dramp.tile([KC, P], F32)

            # ---- merged pass: degree + transpose-all (A read exactly once) ----
            degree_sb = degp.tile([P, MT], F32)  # col mt = degree of rows mt*128..
            for mt in range(MT):
                dcols = smallp.tile([P, NACH], F32, tag="dcols")
                for c in range(NACH):
                    ach = astreamp.tile([P, ACH], F32, tag="ach", bufs=3)
                    nc.sync.dma_start(
                        ach[:], a_d.ap()[mt * P : (mt + 1) * P, c * ACH : (c + 1) * ACH]
                    )
                    achb = astreamp.tile([P, ACH], BF16, tag="achb", bufs=2)
                    nc.scalar.activation(
                        achb[:], ach[:], ACT.Copy, accum_out=dcols[:, c : c + 1]
                    )
                    for g in range(GPC):
                        trp = psA.tile([P, 4 * P], F32, tag="trp")
                        for q in range(4):
                            nc.tensor.matmul(
                                trp[:, q * P : (q + 1) * P],
                                achb[:, (g * 4 + q) * P : (g * 4 + q + 1) * P],
                                identity_bf[:],
                            )
                        kc0 = c * (GPC * 4) + g * 4
                        dst = at_res[
                            :, (mt * KC + kc0) * P : (mt * KC + kc0 + 4) * P
                        ]
                        nc.vector.tensor_copy(dst[:], trp[:])
                nc.vector.reduce_sum(degree_sb[:, mt : mt + 1], dcols[:], axis=AX)

            # ---- ds local, AllGather ds ----
            recip8 = degp.tile([P, MT], F32)
            nc.vector.reciprocal(recip8[:], degree_sb[:])
            dsown = degp.tile([P, MT], F32)
            nc.scalar.activation(dsown[:], recip8[:], ACT.Sqrt)
            dsT_ps = psaux.tile([MT, P], F32, tag="aux")
            nc.tensor.transpose(dsT_ps[:], dsown[:], identity[:])
            dsT_sb = smallp.tile([MT, P], F32, tag="degT")
            nc.vector.tensor_copy(dsT_sb[:], dsT_ps[:])
            nc.sync.dma_start(cin[:], dsT_sb[:])
            nc.gpsimd.collective_compute(
                "AllGather",
                ALU.bypass,
                ins=[cin.opt()],
                outs=[cout.opt()],
                replica_groups=[list(range(NCORES))],
            )
            # keep the PE HAM-warm across the collective stall
            for wi in range(NWARM):
                junk_ps = psA.tile([P, D], F32, tag="trp", name=f"junk{wi}")
                nc.tensor.matmul(junk_ps[:], identity_bf[:], at_res[:, :D])
            # cout row g = global ds of rows [g*128, (g+1)*128)
            dsall_sb = smallp.tile([KC, P], F32, tag="degall")
            nc.sync.dma_start(dsall_sb[:], cout[:])
            dsallT_ps = psaux.tile([P, KC], F32, tag="aux")
            nc.tensor.transpose(dsallT_ps[:], dsall_sb[:], identity[:KC, :KC])
            ds_sb = degp.tile([P, KC], F32)  # ds_sb[p, g] = ds[g*128 + p]
            nc.vector.tensor_copy(ds_sb[:], dsallT_ps[:])

            # ---- main matmul: kc-outer over 2 groups of MTG m-tiles ----
            f_blk = f_d.ap().rearrange("(b c p) d -> b p c d", c=FKC, p=P)
            for mtg in range(MT // MTG):
                ys = [
                    psY.tile([P, D], F32, tag="y", name=f"y{mtg}_{i}")
                    for i in range(MTG)
                ]
                for fb in range(KC // FKC):
                    fch = fstreamp.tile([P, FKC * D], F32, tag="fch")
                    nc.sync.dma_start(
                        fch[:].rearrange("p (c d) -> p c d", c=FKC), f_blk[fb]
                    )
                    for j in range(FKC):
                        kc = fb * FKC + j
                        xp = xpp.tile([P, D], BF16, tag="xp")
                        nc.vector.tensor_scalar_mul(
                            xp[:], fch[:, j * D : (j + 1) * D], ds_sb[:, kc : kc + 1]
                        )
                        for mi in range(MTG):
                            mt = mtg * MTG + mi
                            nc.tensor.matmul(
                                ys[mi][:],
                                at_res[
                                    :, (mt * KC + kc) * P : (mt * KC + kc + 1) * P
                                ],
                                xp[:],
                                start=(kc == 0),
                                stop=(kc == KC - 1),
                            )
                # epilogue per m-tile in the group
                for mi in range(MTG):
                    mt = mtg * MTG + mi
                    fres_t = epip.tile([P, D], F32, tag="fres")
                    nc.scalar.dma_start(
                        fres_t[:], fres_d.ap()[mt * P : (mt + 1) * P, :]
                    )
                    mf = epip.tile([P, D], F32, tag="mf")
                    nc.vector.scalar_tensor_tensor(
                        mf[:],
                        ys[mi][:],
                        dsown[:, mt : mt + 1],
                        fres_t[:],
                        op0=ALU.mult,
                        op1=ALU.add,
                    )
                    o_ps = psA.tile([P, D], F32, tag="trp", name=f"o_ps{mt}")
                    for wc in range(4):
                        mfT_ps = psaux.tile([P, P], F32, tag="aux")
                        nc.tensor.transpose(
                            mfT_ps[:], mf[:, wc * P : (wc + 1) * P], identity[:]
                        )
                        mfT_sb = mftp.tile([P, P], F32R, tag="mfT")
                        nc.vector.tensor_copy(mfT_sb[:], mfT_ps[:])
                        nc.tensor.matmul(
                            o_ps[:],
                            mfT_sb[:],
                            w_sb[:, wc * D : (wc + 1) * D],
                            start=(wc == 0),
                            stop=(wc == 3),
                        )
                    opre = epip.tile([P, D], F32, tag="mf", name="opre")
                    nc.vector.tensor_tensor(
                        opre[:], o_ps[:], b_bcast[:], op=ALU.add
                    )
                    osb = epip.tile([P, D], F32, tag="osb")
                    nc.scalar.activation(osb[:], opre[:], ACT.Relu)
                    nc.scalar.dma_start(out_d.ap()[mt * P : (mt + 1) * P, :], osb[:])

    nc.compile()
    return nc


def _get_nc():
    if "nc" not in _NC_CACHE:
        _NC_CACHE["nc"] = _build()
    return _NC_CACHE["nc"]


def run(inputs, trace=False, trace_kwargs=None):
    """Run the SPMD kernel; returns (full_output, BassKernelResults)."""
    a = np.ascontiguousarray(np.asarray(inputs["adjacency_matrix"], dtype=np.float32))
    f = np.ascontiguousarray(np.asarray(inputs["feature"], dtype=np.float32))
    w = np.ascontiguousarray(np.asarray(inputs["W"], dtype=np.float32))
    b = np.ascontiguousarray(np.asarray(inputs["b"], dtype=np.float32)).reshape(1, D)

    nc = _get_nc()
    in_maps = []
    for d in range(NCORES):
        rows = slice(d * R, (d + 1) * R)
        in_maps.append({"a": a[rows], "f": f, "fres": f[rows], "w": w, "bias": b})
    res = bass_utils.run_bass_kernel_spmd(
        nc,
        in_maps,
        core_ids=list(range(NCORES)),
        trace=trace,
        **(trace_kwargs or {}),
    )
    out = np.concatenate([r["out"] for r in res.results], axis=0)
    return out, res


def kernel(**inputs):
    out, _ = run(inputs, trace=False)
    return out
